# revision 5
# baseline (speedup 1.0000x reference)
"""SIR ODE batch integrator on 8 Trainium2 NeuronCores (Bass/Tile).

Problem: for each of B=65536 samples with params (beta, gamma, S0, I0),
integrate dS=-bSI, dI=bSI-gI, dR=gI over 199 fixed intervals
(t = linspace(0,100,200), fp32) and return the trajectory [B, 200, 3].

v2 design (vs v1's fp32 Euler-per-interval tail):
  - Pure data parallel: 8192 samples per core as [128 part, 64 free].
  - Variables: w = beta*S, m = -beta*I.  Dynamics: dw/dt = w*m,
    dm/dt = m*(w - gamma).  Multiplicative Euler step (size h):
      r = 1 + h*m,  q = (1 - h*gamma) + h*w,  w' = w*r,  m' = m*q
    = ONE stt (qr = swap(y)*h + [1|G]) + ONE tt (y' = y*qr) per step.
  - fp16 state + fp16 ops everywhere after the RK4 head: DVE 2x_1p mode
    gives 2 elem/cycle, and output DMA bytes halve.
  - Schedule (validated vs reference in numpy fp16 simulation):
      k 0..2     RK4 in fp32 [ct|w] form (transient needs accuracy)
      k 3..13    fp16 Euler, 2 substeps of h=dtbar/2
      k 14..21   fp16 Euler, h=dtbar
      k 22..198  fp16 multiplicative AB2, h=3*dtbar: chain states only at
                 k=24,27,...,198 (59 steps, 3 ops each); skipped slots are
                 cubic-Hermite interpolated on the HOST (exact SIR
                 derivatives from neighboring states) - zero device cost.
  - All fp16 sections run as two half-width sample groups interleaved so
    each group's op latency hides under the other group's ops.
  - Host recovers S = w/beta, I = -m/beta, R = 1 - S - I.
"""

import numpy as np

try:
    import concourse.bass as bass
except ImportError:  # pragma: no cover - container default location
    import sys

    sys.path.insert(0, "/opt/trn_rl_repo")
    import concourse.bass as bass

import concourse.bacc as bacc
import concourse.mybir as mybir
from concourse.ap import AP
from concourse.tile import TileContext
from concourse.bass_utils import run_bass_kernel_spmd

F32 = mybir.dt.float32
F16 = mybir.dt.float16
AL = mybir.AluOpType

N_CORES = 8
B = 65536
PER = B // N_CORES  # 8192 samples per core
P = 128
F = PER // P  # 64
HW = F // 2  # 32 cols per half-width group
NUM_T = 200
NI = NUM_T - 1  # 199 intervals

N_RK4 = 3
N2_END = 14  # fp16 Euler n=2 substeps for k in [N_RK4, N2_END)
KC = 22  # coarse AB2 from this interval
SPAN = 3  # coarse AB2 step covers SPAN intervals

DTBAR = float(np.float32(np.float64(100.0) / 199.0))
H_A = float(np.float32(DTBAR / 2))  # n=2 substep
H_B = DTBAR  # fine Euler
H_C = float(np.float32(SPAN * np.float32(DTBAR)))  # coarse AB2
H_C15 = float(np.float32(1.5) * np.float32(H_C))  # AB2 qr scalar

# Bit-exact fp32 dt values of jnp.linspace diffs (only k<3 used, for RK4).
_DT_BITS = [0x3F00A4AA, 0x3F00A4AA, 0x3F00A4AA]
DTS = np.array(_DT_BITS, dtype=np.uint32).view(np.float32)

# chain states written to the main output, in slab-slice order
CHAIN_KS = list(range(N_RK4, KC)) + list(range(KC + SPAN - 1, NI, SPAN))
NSLICES = len(CHAIN_KS)  # 19 + 59 = 78
assert CHAIN_KS[-1] == NI - 1, CHAIN_KS[-1]
CH = 16  # slices per output chunk (one DMA each)
NCHUNK = (NSLICES + CH - 1) // CH  # 5 (last chunk has 14)


def _ap3(tile_ap, offset, stride2, n2, width):
    """[P, n2, width] view of a tile: free dims [[stride2, n2], [1, width]]."""
    return AP(
        tensor=tile_ap.tensor,
        offset=tile_ap.offset + offset,
        ap=[list(tile_ap.ap[0]), [stride2, n2], [1, width]],
    )


def build_nc(reps=1):
    nc = bacc.Bacc(None)
    pin = nc.declare_dram_parameter("pin", [P, 3 * F], F32, isOutput=False)
    gin = nc.declare_dram_parameter("gin", [P, 6 * F], F16, isOutput=False)
    out_head = nc.declare_dram_parameter("out_head", [P, 6 * F], F32, isOutput=True)
    out = nc.declare_dram_parameter("out", [NCHUNK, P, CH * 2 * F], F16, isOutput=True)
    v = nc.vector

    with TileContext(nc) as tc:
        with (
            tc.tile_pool(name="const", bufs=1) as cpool,
            tc.tile_pool(name="slab", bufs=1) as spool,
            tc.tile_pool(name="work", bufs=2) as wpool,
        ):

            def body(_=None):
                pint = cpool.tile([P, 3 * F], F32, tag="pin")
                nc.sync.dma_start(out=pint[:], in_=pin[:])
                gint = cpool.tile([P, 6 * F], F16, tag="gin")
                nc.sync.dma_start(out=gint[:], in_=gin[:])

                # ---- fp32 RK4 head, [ct|w] state, gamma at col 0 of hslab
                # hslab cols: [gamma(F) | o0 | o1 | o2 | s0 | s1 | s2] (2F each)
                hslab = cpool.tile([P, F + 6 * 2 * F], F32, tag="hslab")
                nc.sync.dma_start(out=hslab[:, 0:F], in_=pin[:, 0:F])

                def hbase(j):
                    return F + j * 2 * F

                def g_state(tile_t, base, g):
                    """[ct_g | w_g] two-block view of a [ct|w] slice."""
                    return _ap3(tile_t[:], base + g * HW, F, 2, HW)

                def g_gw(base, g):
                    """[gamma_g | w_g] two-block view (gamma at hslab col 0)."""
                    return _ap3(hslab[:], g * HW, base + F, 2, HW)

                def head_eval_X(base, tag):
                    """Per-group derivative [X_ct|X_w] = v*[gamma|w]."""
                    vts, xts = [], []
                    for g in range(2):
                        vt = wpool.tile([P, HW], F32, tag=f"v{g}", name=f"v{g}")
                        v.tensor_tensor(
                            vt[:],
                            hslab[:, base + g * HW : base + (g + 1) * HW],
                            hslab[:, base + F + g * HW : base + F + (g + 1) * HW],
                            AL.subtract,
                        )
                        vts.append(vt)
                    for g in range(2):
                        xt = wpool.tile([P, 2, HW], F32, tag=f"{tag}{g}", name=f"{tag}{g}")
                        v.scalar_tensor_tensor(
                            xt[:],
                            vts[g][:].unsqueeze(1).broadcast_to([P, 2, HW]),
                            1.0,
                            g_gw(base, g),
                            AL.mult,
                            AL.mult,
                        )
                        xts.append(xt)
                    return xts

                def head_stt(outs, in0s, scalar, in1s):
                    for g in range(2):
                        v.scalar_tensor_tensor(
                            outs[g], in0s[g][:], scalar, in1s[g], AL.mult, AL.add
                        )

                # initial state lives in the pin tile: [gamma | ct0 | w0]
                cur_tile, cur_base = pint, F
                for k in range(N_RK4):
                    h = float(DTS[k])
                    curg = [g_state(cur_tile, cur_base, g) for g in range(2)]
                    X1 = head_eval_X(cur_base, "X1") if cur_tile is hslab else None
                    if X1 is None:
                        # k=0: state in pint; eval X from pint columns
                        vts, X1 = [], []
                        for g in range(2):
                            vt = wpool.tile([P, HW], F32, tag=f"v{g}", name=f"v{g}")
                            v.tensor_tensor(
                                vt[:],
                                pint[:, F + g * HW : F + (g + 1) * HW],
                                pint[:, 2 * F + g * HW : 2 * F + (g + 1) * HW],
                                AL.subtract,
                            )
                            vts.append(vt)
                        for g in range(2):
                            xt = wpool.tile([P, 2, HW], F32, tag=f"X1{g}", name=f"X1{g}")
                            v.scalar_tensor_tensor(
                                xt[:],
                                vts[g][:].unsqueeze(1).broadcast_to([P, 2, HW]),
                                1.0,
                                _ap3(pint[:], g * HW, 2 * F, 2, HW),  # [gamma_g|w_g]
                                AL.mult,
                                AL.mult,
                            )
                            X1.append(xt)
                    s0b, s1b, s2b = hbase(3), hbase(4), hbase(5)
                    head_stt([g_state(hslab, s0b, g) for g in range(2)], X1, -h / 2, curg)
                    X2 = head_eval_X(s0b, "X2")
                    head_stt([g_state(hslab, s1b, g) for g in range(2)], X2, -h / 2, curg)
                    X3 = head_eval_X(s1b, "X3")
                    head_stt([g_state(hslab, s2b, g) for g in range(2)], X3, -h, curg)
                    X4 = head_eval_X(s2b, "X4")
                    A1 = [wpool.tile([P, 2, HW], F32, tag=f"A1{g}", name=f"A1{g}") for g in range(2)]
                    head_stt([a[:] for a in A1], X2, 2.0, [x[:] for x in X1])
                    A2 = [wpool.tile([P, 2, HW], F32, tag=f"A2{g}", name=f"A2{g}") for g in range(2)]
                    head_stt([a[:] for a in A2], X3, 2.0, [a[:] for a in A1])
                    A3 = [wpool.tile([P, 2, HW], F32, tag=f"A3{g}", name=f"A3{g}") for g in range(2)]
                    for g in range(2):
                        v.tensor_tensor(A3[g][:], A2[g][:], X4[g][:], AL.add)
                    nb = hbase(k)
                    head_stt([g_state(hslab, nb, g) for g in range(2)], A3, -h / 6, curg)
                    cur_tile, cur_base = hslab, nb

                # head output DMA (fp32 [ct|w] x3) - off critical path
                nc.sync.dma_start(out=out_head[:], in_=hslab[:, F : F + 6 * F])

                # ---- transition to fp16 [w|m]: w = w, m = w - ct
                etr = cpool.tile([P, 2 * F], F16, tag="etr")
                v.tensor_scalar_mul(
                    etr[:, 0:F], hslab[:, cur_base + F : cur_base + 2 * F], 1.0
                )
                v.tensor_tensor(
                    etr[:, F : 2 * F],
                    hslab[:, cur_base + F : cur_base + 2 * F],
                    hslab[:, cur_base : cur_base + F],
                    AL.subtract,
                )

                # ---- fp16 sections ----
                slabA = spool.tile([P, CH * 2 * F], F16, tag="slabA")
                slabB = spool.tile([P, CH * 2 * F], F16, tag="slabB")
                slabs = [slabA, slabB]

                def slice_loc(j):
                    s = slabs[(j // CH) % 2]
                    return s, (j % CH) * 2 * F

                def y_g(tile_t, base, g):
                    """[w_g | m_g] two-block view of a [w|m] slice."""
                    return _ap3(tile_t[:], base + g * HW, F, 2, HW)

                def y_swap_g(tile_t, base, g):
                    """[m_g | w_g] swapped view."""
                    return _ap3(tile_t[:], base + F + g * HW, -F, 2, HW)

                def gone_g(which, g):
                    """[1_g | G_g] two-block view of gint section `which`."""
                    return _ap3(gint[:], which * 2 * F + g * HW, F, 2, HW)

                def qr_swap(qr_tile):
                    """[q|r]->[r|q] view of a [P, 2, HW] tile (reads swapped)."""
                    return AP(
                        tensor=qr_tile[:].tensor,
                        offset=qr_tile[:].offset + HW,
                        ap=[list(qr_tile[:].ap[0]), [-HW, 2], [1, HW]],
                    )

                def chunk_dma(j):
                    """Fire the chunk DMA when slice j completes it."""
                    if j % CH == CH - 1:
                        c = j // CH
                        s = slabs[c % 2]
                        nc.sync.dma_start(out=out[c], in_=s[:])
                    elif j == NSLICES - 2 and j // CH == (NSLICES - 1) // CH:
                        # early part of the final partial chunk
                        c = (NSLICES - 1) // CH
                        s = slabs[c % 2]
                        n_in = NSLICES - c * CH
                        nc.sync.dma_start(
                            out=out[c][:, 0 : (n_in - 1) * 2 * F],
                            in_=s[:, 0 : (n_in - 1) * 2 * F],
                        )

                def euler_interval(cur, nxt, h, which, nsub, scratch_tag):
                    """nsub fp16 Euler substeps from state `cur`=(tile, base)
                    to slab slice `nxt`=(tile, base); returns nothing."""
                    src = cur
                    for s in range(nsub):
                        qrs = []
                        for g in range(2):
                            qr = wpool.tile(
                                [P, 2, HW], F16, tag=f"qr{g}", name=f"qr{g}"
                            )
                            v.scalar_tensor_tensor(
                                qr[:],
                                y_swap_g(src[0], src[1], g),
                                h,
                                gone_g(which, g),
                                AL.mult,
                                AL.add,
                            )
                            qrs.append(qr)
                        if s == nsub - 1:
                            dstt, dstb = nxt
                        else:
                            st = wpool.tile(
                                [P, 2 * F], F16, tag=scratch_tag, name=scratch_tag
                            )
                            dstt, dstb = st, 0
                        for g in range(2):
                            v.tensor_tensor(
                                y_g(dstt, dstb, g),
                                y_g(src[0], src[1], g),
                                qrs[g][:],
                                AL.mult,
                            )
                        src = (dstt, dstb)

                # k=3..N2_END-1: two substeps each
                cur = (etr, 0)
                for k in range(N_RK4, N2_END):
                    j = CHAIN_KS.index(k)
                    st, sb = slice_loc(j)
                    euler_interval(cur, (st, sb), H_A, 0, 2, "ystep")
                    cur = (st, sb)
                    chunk_dma(j)

                # k=N2_END..KC-1: single Euler steps
                for k in range(N2_END, KC):
                    j = CHAIN_KS.index(k)
                    st, sb = slice_loc(j)
                    euler_interval(cur, (st, sb), H_B, 1, 1, "ystep")
                    cur = (st, sb)
                    chunk_dma(j)

                # coarse AB2: history = state SPAN intervals back
                jprev = CHAIN_KS.index(KC - 1 - SPAN)
                prev = slice_loc(jprev)
                for k in range(KC + SPAN - 1, NI, SPAN):
                    j = CHAIN_KS.index(k)
                    st, sb = slice_loc(j)
                    us = []
                    for g in range(2):
                        u = wpool.tile([P, 2, HW], F16, tag=f"u{g}", name=f"u{g}")
                        v.scalar_tensor_tensor(
                            u[:],
                            y_g(prev[0], prev[1], g),
                            -1.0 / 3.0,
                            y_g(cur[0], cur[1], g),
                            AL.mult,
                            AL.add,
                        )
                        us.append(u)
                    qrs = []
                    for g in range(2):
                        qr = wpool.tile([P, 2, HW], F16, tag=f"qr{g}", name=f"qr{g}")
                        v.scalar_tensor_tensor(
                            qr[:],
                            qr_swap(us[g]),
                            H_C15,
                            gone_g(2, g),
                            AL.mult,
                            AL.add,
                        )
                        qrs.append(qr)
                    for g in range(2):
                        v.tensor_tensor(
                            y_g(st, sb, g),
                            y_g(cur[0], cur[1], g),
                            qrs[g][:],
                            AL.mult,
                        )
                    prev = cur
                    cur = (st, sb)
                    chunk_dma(j)

                # final slice of the last partial chunk
                lastj = NSLICES - 1
                c = lastj // CH
                n_in = NSLICES - c * CH
                if lastj % CH != CH - 1:
                    s = slabs[c % 2]
                    nc.sync.dma_start(
                        out=out[c][:, (n_in - 1) * 2 * F : n_in * 2 * F],
                        in_=s[:, (n_in - 1) * 2 * F : n_in * 2 * F],
                    )

            if reps == 1:
                body()
            else:
                with tc.For_i(0, reps, 1):
                    body()
    nc.finalize()
    return nc


_NC_CACHE = {}


def _pack_inputs(params: np.ndarray) -> list:
    dtb = np.float32(DTBAR)
    in_maps = []
    for c in range(N_CORES):
        sl = params[c * PER : (c + 1) * PER]
        gamma = sl[:, 1].reshape(P, F)
        pin = np.empty((P, 3 * F), dtype=np.float32)
        pin[:, 0:F] = gamma
        beta = sl[:, 0]
        pin[:, F : 2 * F] = (beta * (sl[:, 2] + sl[:, 3])).reshape(P, F)  # ct0
        pin[:, 2 * F : 3 * F] = (beta * sl[:, 2]).reshape(P, F)  # w0
        gin = np.empty((P, 6 * F), dtype=np.float16)
        one = np.float32(1)
        for i, h in enumerate((np.float32(H_A), np.float32(H_B), np.float32(H_C))):
            gin[:, 2 * i * F : (2 * i + 1) * F] = one
            gin[:, (2 * i + 1) * F : (2 * i + 2) * F] = (one - h * gamma).astype(
                np.float16
            )
        in_maps.append({"pin": pin, "gin": gin})
    return in_maps


def kernel(params: np.ndarray) -> np.ndarray:
    params = np.asarray(params, dtype=np.float32)
    assert params.shape == (B, 4)

    if "nc" not in _NC_CACHE:
        _NC_CACHE["nc"] = build_nc()
    nc = _NC_CACHE["nc"]

    in_maps = _pack_inputs(params)
    res = run_bass_kernel_spmd(nc, in_maps, list(range(N_CORES)))

    f32 = np.float32
    out_full = np.empty((B, NUM_T, 3), dtype=f32)
    one = f32(1)
    beta = params[:, 0]
    gamma = params[:, 1]
    S0 = params[:, 2]
    I0 = params[:, 3]
    ib = (one / beta).reshape(B, 1)
    out_full[:, 0, 0] = S0
    out_full[:, 0, 1] = I0
    out_full[:, 0, 2] = (one - S0) - I0

    # gather per-core outputs into full-B arrays
    # head: [P, 6F] fp32 = 3 slices of [ct(F) | w(F)]
    head_ct = np.empty((3, B), dtype=f32)
    head_w = np.empty((3, B), dtype=f32)
    chain_w = np.empty((NSLICES, B), dtype=f32)
    chain_m = np.empty((NSLICES, B), dtype=f32)
    for c in range(N_CORES):
        oh = res.results[c]["out_head"]  # [P, 6F] fp32
        oh = oh.reshape(P, 3, 2, F)
        bsl = slice(c * PER, (c + 1) * PER)
        for k in range(3):
            head_ct[k, bsl] = oh[:, k, 0, :].reshape(PER)
            head_w[k, bsl] = oh[:, k, 1, :].reshape(PER)
        om = res.results[c]["out"]  # [NCHUNK, P, CH*2F] fp16
        om = om.reshape(NCHUNK, P, CH, 2, F).transpose(0, 2, 1, 3, 4)
        om = om.reshape(NCHUNK * CH, P, 2, F)[:NSLICES].astype(f32)
        chain_w[:, bsl] = om[:, :, 0, :].reshape(NSLICES, PER)
        chain_m[:, bsl] = om[:, :, 1, :].reshape(NSLICES, PER)

    # head intervals 0..2: [ct|w] -> S, I, R
    for k in range(3):
        S = head_w[k] * ib[:, 0]
        C = head_ct[k] * ib[:, 0]
        out_full[:, k + 1, 0] = S
        out_full[:, k + 1, 1] = C - S
        out_full[:, k + 1, 2] = one - C

    # chain states: [w|m] -> S = w/beta, I = -m/beta
    for j, k in enumerate(CHAIN_KS):
        S = chain_w[j] * ib[:, 0]
        I = -chain_m[j] * ib[:, 0]
        out_full[:, k + 1, 0] = S
        out_full[:, k + 1, 1] = I
        out_full[:, k + 1, 2] = (one - S) - I

    # host Hermite interpolation for skipped slots in the coarse section.
    # anchors: k = KC-1, KC+SPAN-1, ..., NI-1 (chain states)
    anchor_js = [CHAIN_KS.index(KC - 1)] + [
        CHAIN_KS.index(k) for k in range(KC + SPAN - 1, NI, SPAN)
    ]
    Wa = chain_w[anchor_js]  # [NA, B]
    Ma = chain_m[anchor_js]
    Fw = Wa * Ma
    Fm = Ma * (Wa - gamma[None, :])
    H = f32(SPAN * DTBAR)
    for d in range(1, SPAN):
        s = f32(d / SPAN)
        h00 = f32(2 * s**3 - 3 * s**2 + 1)
        h10 = f32(s**3 - 2 * s**2 + s)
        h01 = f32(-2 * s**3 + 3 * s**2)
        h11 = f32(s**3 - s**2)
        Wi = h00 * Wa[:-1] + (h10 * H) * Fw[:-1] + h01 * Wa[1:] + (h11 * H) * Fw[1:]
        Mi = h00 * Ma[:-1] + (h10 * H) * Fm[:-1] + h01 * Ma[1:] + (h11 * H) * Fm[1:]
        ks = np.arange(KC - 1 + d, NI - 1, SPAN)  # interval indices for this d
        S = Wi * ib[:, 0][None, :]
        I = -Mi * ib[:, 0][None, :]
        out_full[:, ks + 1, 0] = S.T
        out_full[:, ks + 1, 1] = I.T
        out_full[:, ks + 1, 2] = (one - S.T) - I.T
    return out_full


if __name__ == "__main__":
    rng = np.random.RandomState(0)
    p = rng.uniform(0, 1, (B, 4)).astype(np.float32)
    r = kernel(p)
    print(r.shape, r.dtype, r[0, :3], flush=True)
